# revision 1
# baseline (speedup 1.0000x reference)
"""BertAttention (QKV proj + MHA + output proj + residual + LayerNorm) on 8 TRN2 NeuronCores.

Sharding: batch (4-way) x query-sequence-half (2-way) => 8 shards, no collectives.
Core c handles batch b=c//2, query half c%2. Each core computes K/V for its full
batch sequence (all heads) and Q/attention/output-proj/LayerNorm for its 1024
query rows. K/V projection work is duplicated across the 2 cores sharing a batch;
in exchange there is zero cross-core communication.

The host permutes each core's X rows so its query half comes first — attention is
permutation-invariant over keys as long as (K, V, mask) share the permutation, so
the program is identical across cores (pure SPMD) with no per-core indices.

Layouts (SBUF partition dim first):
  Xt, Kt:  [128, H/128, S]   transposed activations (feature on partitions), bf16
  Qt:      [128, H/128, SH]  transposed, bf16
  V:       [128, S/128, NH*65] natural ([tok, head-dim]) with a ones column per
           head at slot 64 — the PV matmul then yields sum(exp) as row 64 for free
  scores:  St[ktok, qtok] in PSUM; softmax sum over ktok (the partition dim) comes
           from the ones-column trick; max-subtraction safely skipped (|s| <~ 1)
  ctx:     [128, NH/2, SH]   transposed (head dim on partitions), bf16
  out:     natural [qtok, H] — residual add + LayerNorm along the free dim.
"""

from contextlib import ExitStack

import numpy as np

import bass_rust
import concourse.bass as bass
import concourse.mybir as mybir
from concourse.tile import TileContext
from concourse.bass_utils import run_bass_kernel_spmd
from concourse.masks import make_identity

FP = mybir.dt.float32
BF = mybir.dt.bfloat16
AF = mybir.ActivationFunctionType
OP = mybir.AluOpType

N_CORES = 8
EPS = 1e-12

# The walrus build in this toolchain rejects instructions that carry more than
# one sync-wait command ("Too many sync wait commands", CoreV2/V3 setupSyncWait),
# while Tile freely attaches several semaphore waits to one instruction (and the
# TileContext exit drain aggregates one wait per logical processor). Hoist the
# excess waits onto standalone InstEventSemaphore carriers on the same engine,
# placed immediately before the instruction — engine streams are serial, so the
# gating semantics are identical.
_MAX_WAITS_PER_INST = 1


def _split_sync_waits(nc, cap=_MAX_WAITS_PER_INST):
    n_split = 0
    for fn in nc.m.functions:
        for bb in fn.blocks:
            insts = list(bb.instructions)
            out = []
            changed = False
            for ins in insts:
                si = ins.sync_info
                waits = list(si.on_wait) if (si is not None and si.on_wait) else []
                if len(waits) > cap:
                    head, tail = waits[: len(waits) - cap], waits[len(waits) - cap :]
                    for j, w in enumerate(head):
                        ev = mybir.InstEventSemaphore(
                            name=f"{ins.name}-sw{j}",
                            engine=ins.engine,
                            ins=[],
                            outs=[],
                            sync_info=bass_rust.SyncInfo(on_wait=[w], on_update=[]),
                        )
                        out.append(ev)
                        n_split += 1
                    si.on_wait = tail
                    changed = True
                out.append(ins)
            if changed:
                bb.instructions[:] = out
    return n_split


def _dram_row_bcast(handle, p, n):
    """AP reading DRAM vector [n] broadcast across p partitions."""
    return bass.AP(tensor=handle, offset=0, ap=[[0, p], [1, n]])


def _build(s, h, nh, sh, flags, split=True, stop_after=None):
    """Build the per-core Bass program. flags: which bias/affine inputs matter."""
    hd = h // nh
    assert hd == 64, "head packing assumes head_dim 64 (2 heads per 128 partitions)"
    kt_n = h // 128  # contraction tiles over hidden dim
    tt_n = s // 128  # key-token tiles
    qt_n = sh // 128  # query-token tiles
    qc = min(512, sh)  # matmul moving-dim chunk over query tokens
    scale = 1.0 / float(np.sqrt(hd))

    nc = bass.Bass(target_bir_lowering=False)
    x = nc.dram_tensor("x", [s, h], FP, kind="ExternalInput")
    mask = nc.dram_tensor("mask", [s], FP, kind="ExternalInput")
    w_dram = {
        n: nc.dram_tensor(n, [h, h], FP, kind="ExternalInput")
        for n in ("wq", "wk", "wv", "wo")
    }
    vec_dram = {
        n: nc.dram_tensor(n, [h], FP, kind="ExternalInput")
        for n in ("bq", "bk", "bv", "bo", "ln_gamma", "ln_beta")
        if flags[n]
    }
    out = nc.dram_tensor("out", [sh, h], FP, kind="ExternalOutput")

    with TileContext(nc) as tc, ExitStack() as st_all:
        persist = st_all.enter_context(tc.tile_pool(name="persist", bufs=1))
        dram = st_all.enter_context(tc.tile_pool(name="dram", bufs=1, space="DRAM"))
        st_mid = st_all.enter_context(ExitStack())
        # attention-phase SBUF pools allocated low in the stack so they do not
        # overlap the released weight/X zones (which would serialize phases)
        psb = st_mid.enter_context(tc.tile_pool(name="psb", bufs=2))
        rpool = st_mid.enter_context(tc.tile_pool(name="rpool", bufs=2))

        qt = persist.tile([128, kt_n, sh], BF)
        kt = persist.tile([128, kt_n, s], BF)
        vsb = persist.tile([128, tt_n, nh * 65], BF)
        ctx_t = persist.tile([128, nh // 2, sh], BF)
        wo_bf = persist.tile([128, kt_n, h], BF)
        mask_sb = persist.tile([128, tt_n], FP)
        eps_sb = persist.tile([128, 1], FP)

        nc.vector.memset(eps_sb, EPS)
        nc.sync.dma_start(out=mask_sb, in_=mask[:].rearrange("(t p) -> p t", p=128))

        # bias columns for Qt/Kt evictions (partition = output feature in tile)
        bias_cols = {}
        for name in ("bq", "bk"):
            if flags[name]:
                col = persist.tile([128, kt_n], FP, name=f"{name}_col")
                nc.sync.dma_start(
                    out=col, in_=vec_dram[name][:].rearrange("(t p) -> p t", p=128)
                )
                bias_cols[name] = col
        # rows broadcast across partitions for V/out bias and LN affine
        bcast = {}
        for name in ("bv", "bo", "ln_gamma", "ln_beta"):
            if flags[name]:
                t = persist.tile([128, h], FP, name=f"{name}_bc")
                nc.sync.dma_start(out=t, in_=_dram_row_bcast(vec_dram[name], 128, h))
                bcast[name] = t

        # ones columns in V (slot 64 of each 65-wide head block)
        for m in range(tt_n):
            v_view = vsb[:, m, :].rearrange("p (a e) -> p a e", e=65)
            nc.vector.memset(v_view[:, :, 64:65], 1.0)

        ident = persist.tile([128, 128], BF)
        make_identity(nc, ident)

        with ExitStack() as st_proj:
            xtpool = st_proj.enter_context(tc.tile_pool(name="xtpool", bufs=1))
            wbuf = st_proj.enter_context(tc.tile_pool(name="wbuf", bufs=2))
            st_pp = st_proj.enter_context(ExitStack())
            stage = st_pp.enter_context(tc.tile_pool(name="stage", bufs=2))
            projps = st_pp.enter_context(
                tc.tile_pool(name="projps", bufs=2, space="PSUM")
            )
            tps = st_pp.enter_context(tc.tile_pool(name="tps", bufs=4, space="PSUM"))

            xt = xtpool.tile([128, kt_n, s], BF)

            hc = min(512, h)  # staging chunk (SBUF pressure)

            def load_weight(dname, w_bf=None):
                if w_bf is None:
                    w_bf = wbuf.tile([128, kt_n, h], BF, name=f"{dname}_bf", tag="w")
                for k in range(kt_n):
                    for c0 in range(0, h, hc):
                        stg = stage.tile([128, hc], FP, name="wstg", tag="stg")
                        nc.sync.dma_start(
                            out=stg, in_=w_dram[dname][k * 128 : (k + 1) * 128, c0 : c0 + hc]
                        )
                        nc.vector.tensor_copy(out=w_bf[:, k, c0 : c0 + hc], in_=stg)
                return w_bf

            # X: load f32, cast bf16, transpose on the (otherwise idle) TensorE —
            # this also pre-warms the PE clock gate before the projections.
            for t in range(tt_n):
                xbt = stage.tile([128, h], BF, name="xbt", tag="xbt")
                for c0 in range(0, h, hc):
                    stg = stage.tile([128, hc], FP, name="xstg", tag="stg")
                    nc.sync.dma_start(out=stg, in_=x[t * 128 : (t + 1) * 128, c0 : c0 + hc])
                    nc.vector.tensor_copy(out=xbt[:, c0 : c0 + hc], in_=stg)
                for k in range(kt_n):
                    tp = tps.tile([128, 128], BF, name="tp")
                    nc.tensor.transpose(tp, xbt[:, k * 128 : (k + 1) * 128], ident)
                    nc.scalar.activation(
                        out=xt[:, k, t * 128 : (t + 1) * 128], in_=tp, func=AF.Copy
                    )

            def kq_group(w_bf, dst, bias_col, m, n0, pool):
                """One K/Q projection PSUM group: 8 accumulating matmuls + evict."""
                n1 = min(n0 + 512, dst.shape[2])
                ps = pool.tile([128, 512], FP, name="projp", tag="projp")
                for k in range(kt_n):
                    nc.tensor.matmul(
                        ps[:, : n1 - n0],
                        w_bf[:, k, m * 128 : (m + 1) * 128],
                        xt[:, k, n0:n1],
                        start=(k == 0),
                        stop=(k == kt_n - 1),
                    )
                if bias_col is not None:
                    nc.vector.tensor_scalar_add(
                        out=dst[:, m, n0:n1],
                        in0=ps[:, : n1 - n0],
                        scalar1=bias_col[:, m : m + 1],
                    )
                else:
                    nc.vector.tensor_copy(out=dst[:, m, n0:n1], in_=ps[:, : n1 - n0])

            def v_group(wv_bf, m, n0):
                ps = projps.tile([128, 512], FP, name="projp", tag="projp")
                for k in range(kt_n):
                    nc.tensor.matmul(
                        ps,
                        xt[:, k, m * 128 : (m + 1) * 128],
                        wv_bf[:, k, n0 : n0 + 512],
                        start=(k == 0),
                        stop=(k == kt_n - 1),
                    )
                dst = vsb[:, m, :].rearrange("p (a e) -> p a e", e=65)[
                    :, n0 // 64 : n0 // 64 + 8, 0:64
                ]
                src = ps.rearrange("p (a e) -> p a e", e=64)
                if "bv" in bcast:
                    nc.vector.tensor_add(
                        out=dst,
                        in0=src,
                        in1=bcast["bv"][:, n0 : n0 + 512].rearrange(
                            "p (a e) -> p a e", e=64
                        ),
                    )
                else:
                    nc.vector.tensor_copy(out=dst, in_=src)

            # upfront: V (all chunks, chunk-major so early heads unblock first),
            # then K/Q for the first few feature tiles; the rest of K/Q is
            # interleaved into the attention loop as PE gap-filler.
            wv_bf = load_weight("wv")
            wk_bf = load_weight("wk")
            for n0 in range(0, h, 512):
                for m in range(tt_n):
                    v_group(wv_bf, m, n0)
            wq_bf = load_weight("wq")
            load_weight("wo", w_bf=wo_bf)

            n_up = 2 if kt_n <= 4 else 4
            for m in range(n_up):
                for n0 in range(0, s, 512):
                    kq_group(wk_bf, kt, bias_cols.get("bk"), m, n0, projps)
                for n0 in range(0, sh, 512):
                    kq_group(wq_bf, qt, bias_cols.get("bq"), m, n0, projps)

            fill_tasks = []
            for m in range(n_up, kt_n):
                for n0 in range(0, s, 512):
                    fill_tasks.append(("k", m, n0))
                for n0 in range(0, sh, 512):
                    fill_tasks.append(("q", m, n0))
            # pacing: tile m's groups must land before head 2m-1 starts
            pace = 10**9
            gidx = 0
            for m in range(n_up, kt_n):
                gidx += s // 512 + max(1, sh // 512)
                deadline = tt_n * max(1, 2 * m - 1)
                pace = min(pace, max(1, deadline // gidx))

            st_pp.close()  # release stage (SBUF) + projps/tps (PSUM)

            # ---- attention, with projection fill interleaved ----
            with (
                tc.tile_pool(name="stps", bufs=2, space="PSUM") as stps,
                tc.tile_pool(name="pvps", bufs=1, space="PSUM") as pvps,
                tc.tile_pool(name="fillps", bufs=2, space="PSUM") as fillps,
                tc.tile_pool(name="ctxu", bufs=2) as ctxu_pool,
            ):
                it = 0
                for hh in range(nh if stop_after != "proj" else 0):
                    mt, po = hh // 2, 64 * (hh % 2)
                    pv = pvps.tile([65, sh], FP, name="pvp")
                    for m in range(tt_n):
                        stt = stps.tile([128, sh], FP, name="stp")
                        for n0 in range(0, sh, qc):
                            nc.tensor.matmul(
                                stt[:, n0 : n0 + qc],
                                kt[po : po + 64, mt, m * 128 : (m + 1) * 128],
                                qt[po : po + 64, mt, n0 : n0 + qc],
                                start=True,
                                stop=True,
                            )
                        p = psb.tile([128, sh], BF, name="pexp")
                        nc.scalar.activation(
                            p, stt, AF.Exp, bias=mask_sb[:, m : m + 1], scale=scale
                        )
                        for n0 in range(0, sh, qc):
                            nc.tensor.matmul(
                                pv[:, n0 : n0 + qc],
                                vsb[:, m, hh * 65 : (hh + 1) * 65],
                                p[:, n0 : n0 + qc],
                                start=(m == 0),
                                stop=(m == tt_n - 1),
                            )
                        it += 1
                        if fill_tasks and it % pace == 0:
                            kind, fm, fn0 = fill_tasks.pop(0)
                            if kind == "k":
                                kq_group(wk_bf, kt, bias_cols.get("bk"), fm, fn0, fillps)
                            else:
                                kq_group(wq_bf, qt, bias_cols.get("bq"), fm, fn0, fillps)
                    # quick-free eviction: copy + reciprocal release the PV bank;
                    # the broadcast/normalize chain completes out of line.
                    ctx_u = ctxu_pool.tile([64, sh], BF, name="ctxu")
                    nc.vector.tensor_copy(out=ctx_u, in_=pv[0:64, :])
                    r = rpool.tile([1, sh], FP, name="recip")
                    nc.vector.reciprocal(r, pv[64:65, :])
                    # broadcast r across 64 partitions via a DRAM roundtrip (DMA
                    # partition-broadcast needs a DRAM source on this toolchain)
                    r_dram = dram.tile([sh], FP, name="rdram", tag="rdram", bufs=2)
                    nc.sync.dma_start(out=r_dram, in_=r)
                    rbc = rpool.tile([64, sh], FP, name="recipbc", bufs=2)
                    nc.sync.dma_start(
                        out=rbc,
                        in_=bass.AP(
                            tensor=r_dram.tensor,
                            offset=r_dram.offset,
                            ap=[[0, 64], [1, sh]],
                        ),
                    )
                    nc.vector.tensor_mul(
                        out=ctx_t[po : po + 64, mt, :], in0=ctx_u, in1=rbc
                    )
                for kind, fm, fn0 in fill_tasks:  # leftovers (debug paths)
                    if kind == "k":
                        kq_group(wk_bf, kt, bias_cols.get("bk"), fm, fn0, fillps)
                    else:
                        kq_group(wq_bf, qt, bias_cols.get("bq"), fm, fn0, fillps)

        st_mid.close()  # release psb/rpool before output phase

        # ---- output projection + residual + LayerNorm (natural layout) ----
        with (
            tc.tile_pool(name="ops", bufs=4, space="PSUM") as ops,
            tc.tile_pool(name="osb", bufs=2) as osb,
            tc.tile_pool(name="lnp", bufs=2) as lnp,
        ):
            for m in range(qt_n if stop_after in (None, 'oproj') else 0):
                pss = []
                for n0 in range(0, h, 512):
                    ps = ops.tile([128, 512], FP, name="op")
                    # ctx_t tile mt holds heads 2mt / 2mt+1 on partitions
                    # 0-63 / 64-127, exactly matching Wo rows mt*128..(mt+1)*128,
                    # so one K=128 matmul contracts both heads at once.
                    for mt in range(nh // 2):
                        nc.tensor.matmul(
                            ps,
                            ctx_t[:, mt, m * 128 : (m + 1) * 128],
                            wo_bf[:, mt, n0 : n0 + 512],
                            start=(mt == 0),
                            stop=(mt == nh // 2 - 1),
                        )
                    pss.append((n0, ps))
                xres = osb.tile([128, h], FP, name="xres")
                nc.sync.dma_start(out=xres, in_=x[m * 128 : (m + 1) * 128, :])
                o = osb.tile([128, h], FP, name="osum")
                for n0, ps in pss:
                    nc.vector.tensor_add(
                        out=o[:, n0 : n0 + 512], in0=ps, in1=xres[:, n0 : n0 + 512]
                    )
                if "bo" in bcast:
                    nc.vector.tensor_add(out=o, in0=o, in1=bcast["bo"])
                if stop_after == "oproj":
                    nc.sync.dma_start(out=out[m * 128 : (m + 1) * 128, :], in_=o)
                    continue
                nsub = (h + 511) // 512
                stats = lnp.tile([128, nsub, 6], FP, name="stats")
                for i in range(nsub):
                    nc.vector.bn_stats(
                        out=stats[:, i, :], in_=o[:, i * 512 : (i + 1) * 512]
                    )
                mv = lnp.tile([128, 2], FP, name="mv")
                nc.vector.bn_aggr(out=mv, in_=stats)
                std = lnp.tile([128, 1], FP, name="std")
                nc.scalar.activation(std, mv[:, 1:2], AF.Sqrt, bias=eps_sb)
                inv = lnp.tile([128, 1], FP, name="inv")
                nc.vector.reciprocal(inv, std)
                y = osb.tile([128, h], FP, name="yout")
                nc.vector.tensor_scalar(
                    out=y,
                    in0=o,
                    scalar1=mv[:, 0:1],
                    scalar2=inv,
                    op0=OP.subtract,
                    op1=OP.mult,
                )
                if "ln_gamma" in bcast:
                    nc.vector.tensor_mul(out=y, in0=y, in1=bcast["ln_gamma"])
                if "ln_beta" in bcast:
                    nc.vector.tensor_add(out=y, in0=y, in1=bcast["ln_beta"])
                nc.sync.dma_start(out=out[m * 128 : (m + 1) * 128, :], in_=y)
            if stop_after not in (None, 'oproj'):
                for m in range(qt_n):
                    dbg = osb.tile([128, h], FP, name="dbg", tag="xres")
                    if stop_after == "proj":
                        nc.vector.tensor_copy(out=dbg, in_=kt[:, 0, 0:h])
                    else:
                        nc.vector.tensor_copy(out=dbg[0:64, :sh], in_=ctx_t[0:64, 0, :])
                        nc.vector.tensor_copy(out=dbg[64:128, :sh], in_=ctx_t[64:128, 0, :])
                    nc.sync.dma_start(out=out[m * 128 : (m + 1) * 128, :], in_=dbg)

    if split:
        _split_sync_waits(nc)
    return nc


_NC_CACHE = {}


def _get_nc(s, h, nh, sh, flags):
    key = (s, h, nh, sh, tuple(sorted(flags.items())))
    if key not in _NC_CACHE:
        _NC_CACHE[key] = _build(s, h, nh, sh, flags)
    return _NC_CACHE[key]


def _prepare(hidden_states, attention_mask, Wq, bq, Wk, bk, Wv, bv, Wo, bo, ln_gamma, ln_beta):
    hs = np.ascontiguousarray(np.asarray(hidden_states, dtype=np.float32))
    b_, s_, h_ = hs.shape
    nh_ = h_ // 64
    sh_ = s_ // 2
    am = np.asarray(attention_mask, dtype=np.float32).reshape(b_, s_)
    flags = {
        "bq": bool(np.any(np.asarray(bq))),
        "bk": bool(np.any(np.asarray(bk))),
        "bv": bool(np.any(np.asarray(bv))),
        "bo": bool(np.any(np.asarray(bo))),
        "ln_gamma": not bool(np.all(np.asarray(ln_gamma) == 1.0)),
        "ln_beta": bool(np.any(np.asarray(ln_beta))),
    }
    nc = _get_nc(s_, h_, nh_, sh_, flags)

    f32c = lambda a: np.ascontiguousarray(np.asarray(a, dtype=np.float32))
    shared = {"wq": f32c(Wq), "wk": f32c(Wk), "wv": f32c(Wv), "wo": f32c(Wo)}
    for name, arr in (
        ("bq", bq),
        ("bk", bk),
        ("bv", bv),
        ("bo", bo),
        ("ln_gamma", ln_gamma),
        ("ln_beta", ln_beta),
    ):
        if flags[name]:
            shared[name] = f32c(arr)

    in_maps = []
    for c in range(N_CORES):
        bb, half = c // 2, c % 2
        mine = slice(half * sh_, (half + 1) * sh_)
        other = slice((1 - half) * sh_, (2 - half) * sh_)
        xp = np.ascontiguousarray(np.concatenate([hs[bb, mine], hs[bb, other]], axis=0))
        mp = np.ascontiguousarray(np.concatenate([am[bb, mine], am[bb, other]]))
        in_maps.append({"x": xp, "mask": mp, **shared})
    return nc, in_maps, (b_, s_, h_, sh_)


def _assemble(results, shape):
    b_, s_, h_, sh_ = shape
    out = np.empty((b_, s_, h_), dtype=np.float32)
    for c in range(N_CORES):
        bb, half = c // 2, c % 2
        out[bb, half * sh_ : (half + 1) * sh_] = results[c]["out"]
    return out


def kernel(**inputs) -> np.ndarray:
    nc, in_maps, shape = _prepare(**inputs)
    res = run_bass_kernel_spmd(nc, in_maps, core_ids=list(range(N_CORES)))
    return _assemble(res.results, shape)



# revision 9
# speedup vs baseline: 1.3534x; 1.3534x over previous
"""BertAttention (QKV proj + MHA + output proj + residual + LayerNorm) on 8 TRN2 NeuronCores.

Sharding: batch (4-way) x query-sequence-half (2-way) => 8 shards, no collectives.
Core c handles batch b=c//2, query half c%2. Each core computes K/V for its full
batch sequence (all heads) and Q/attention/output-proj/LayerNorm for its 1024
query rows. K/V projection work is duplicated across the 2 cores sharing a batch;
in exchange there is zero cross-core communication.

The host permutes each core's X rows so its query half comes first — attention is
permutation-invariant over keys as long as (K, V, mask) share the permutation, so
the program is identical across cores (pure SPMD) with no per-core indices.

Host pre-stages inputs: X transposed to [H, S] bf16 (feature on partitions after
DMA), weights cast to bf16, residual rows kept fp32. This removes all on-device
casts and PE transposes and halves the load DMA bytes.

Layouts (SBUF partition dim first):
  xt:      [128, H/128, S]   transposed activations, bf16 (direct DMA)
  Kt:      [128, H/128, S]   transposed keys (feature on partitions), bf16
  Qt:      [128, H/128, SH]  transposed, bf16
  V:       [128, S/128, NH*65] natural ([tok, head-dim]) with a ones column per
           head at slot 64 — the PV matmul then yields sum(exp) as row 64 for free
  scores:  St[ktok, qtok] in PSUM; softmax sum over ktok (the partition dim) comes
           from the ones-column trick; max-subtraction safely skipped (|s| <~ 1)
  ctx:     [128, NH/2, SH]   transposed (head dim on partitions), bf16
  out:     natural [qtok, H] — residual add + LayerNorm along the free dim.

Schedule: K/Q tile 0 + V chunk 0 projected upfront (~25us, overlapped with the
weight DMA), then the 16-head attention loop starts; remaining projection groups
are deadline-paced into the loop (one group per scores iteration) as PE
gap-filler. PV PSUM accumulators ping-pong across heads so the softmax-sum
eviction (DMA roundtrip broadcast + reciprocal + multiply) never stalls the PE.
"""

from contextlib import ExitStack

import numpy as np
import ml_dtypes

import bass_rust
import concourse.bass as bass
import concourse.mybir as mybir
from concourse.tile import TileContext
from concourse.bass_utils import run_bass_kernel_spmd

FP = mybir.dt.float32
BF = mybir.dt.bfloat16
AF = mybir.ActivationFunctionType
OP = mybir.AluOpType

N_CORES = 8
EPS = 1e-12

# The walrus build in this toolchain rejects instructions that carry more than
# one sync-wait command ("Too many sync wait commands", CoreV2/V3 setupSyncWait),
# while Tile freely attaches several semaphore waits to one instruction (and the
# TileContext exit drain aggregates one wait per logical processor). Hoist the
# excess waits onto standalone InstEventSemaphore carriers on the same engine,
# placed immediately before the instruction — engine streams are serial, so the
# gating semantics are identical.
_MAX_WAITS_PER_INST = 1


def _split_sync_waits(nc, cap=_MAX_WAITS_PER_INST):
    n_split = 0
    for fn in nc.m.functions:
        for bb in fn.blocks:
            insts = list(bb.instructions)
            out = []
            changed = False
            for ins in insts:
                si = ins.sync_info
                waits = list(si.on_wait) if (si is not None and si.on_wait) else []
                if len(waits) > cap:
                    head, tail = waits[: len(waits) - cap], waits[len(waits) - cap :]
                    for j, w in enumerate(head):
                        ev = mybir.InstEventSemaphore(
                            name=f"{ins.name}-sw{j}",
                            engine=ins.engine,
                            ins=[],
                            outs=[],
                            sync_info=bass_rust.SyncInfo(on_wait=[w], on_update=[]),
                        )
                        out.append(ev)
                        n_split += 1
                    si.on_wait = tail
                    changed = True
                out.append(ins)
            if changed:
                bb.instructions[:] = out
    return n_split


def _dram_row_bcast(handle, p, n):
    """AP reading DRAM vector [n] broadcast across p partitions."""
    return bass.AP(tensor=handle, offset=0, ap=[[0, p], [1, n]])


def _build(s, h, nh, sh, flags):
    """Build the per-core Bass program. flags: which bias/affine inputs matter."""
    hd = h // nh
    assert hd == 64, "head packing assumes head_dim 64 (2 heads per 128 partitions)"
    kt_n = h // 128  # contraction tiles over hidden dim
    tt_n = s // 128  # key-token tiles
    qt_n = sh // 128  # query-token tiles
    scale = 1.0 / float(np.sqrt(hd))

    nc = bass.Bass(target_bir_lowering=False)
    x = nc.dram_tensor("x", [sh, h], FP, kind="ExternalInput")  # residual rows
    xt_d = nc.dram_tensor("xt", [h, s], BF, kind="ExternalInput")  # transposed
    mask = nc.dram_tensor("mask", [s], FP, kind="ExternalInput")
    w_dram = {
        n: nc.dram_tensor(n, [h, h], BF, kind="ExternalInput")
        for n in ("wq", "wk", "wv", "wo")
    }
    vec_dram = {
        n: nc.dram_tensor(n, [h], FP, kind="ExternalInput")
        for n in ("bq", "bk", "bv", "bo", "ln_gamma", "ln_beta")
        if flags[n]
    }
    out = nc.dram_tensor("out", [sh, h], FP, kind="ExternalOutput")

    with TileContext(nc) as tc, ExitStack() as st_all:
        persist = st_all.enter_context(tc.tile_pool(name="persist", bufs=1))
        dram = st_all.enter_context(tc.tile_pool(name="dram", bufs=1, space="DRAM"))
        qt = persist.tile([128, kt_n, sh], BF)
        kt = persist.tile([128, kt_n, s], BF)
        vsb = persist.tile([128, tt_n, nh * 65], BF)
        ctx_t = persist.tile([128, nh // 2, sh], BF)
        mask_sb = persist.tile([128, tt_n], FP)
        eps_sb = persist.tile([128, 1], FP)

        nc.vector.memset(eps_sb, EPS)
        nc.sync.dma_start(out=mask_sb, in_=mask[:].rearrange("(t p) -> p t", p=128))

        # bias columns for Qt/Kt evictions (partition = output feature in tile)
        bias_cols = {}
        for name in ("bq", "bk"):
            if flags[name]:
                col = persist.tile([128, kt_n], FP, name=f"{name}_col")
                nc.sync.dma_start(
                    out=col, in_=vec_dram[name][:].rearrange("(t p) -> p t", p=128)
                )
                bias_cols[name] = col
        # rows broadcast across partitions for V/out bias and LN affine
        bcast = {}
        for name in ("bv", "bo", "ln_gamma", "ln_beta"):
            if flags[name]:
                t = persist.tile([128, h], FP, name=f"{name}_bc")
                nc.sync.dma_start(out=t, in_=_dram_row_bcast(vec_dram[name], 128, h))
                bcast[name] = t

        # ones columns in V (slot 64 of each 65-wide head block)
        for m in range(tt_n):
            v_view = vsb[:, m, :].rearrange("p (a e) -> p a e", e=65)
            nc.vector.memset(v_view[:, :, 64:65], 1.0)

        with ExitStack() as st_proj:
            xtpool = st_proj.enter_context(tc.tile_pool(name="xtpool", bufs=1))
            wbuf = st_proj.enter_context(tc.tile_pool(name="wbuf", bufs=3))

            xt = xtpool.tile([128, kt_n, s], BF)

            # slot assignment: wv takes slot 0 so the late wo load (issued
            # after the last V fill group) can reuse it; wk/wq live through
            # the whole attention loop (K/Q fill groups read them).
            wv_bf = wbuf.tile([128, kt_n, h], BF, name="wv_bf", tag="w")
            wk_bf = wbuf.tile([128, kt_n, h], BF, name="wk_bf", tag="w")
            wq_bf = wbuf.tile([128, kt_n, h], BF, name="wq_bf", tag="w")

            def load_w(dname, dst):
                for k in range(kt_n):
                    nc.sync.dma_start(
                        out=dst[:, k, :], in_=w_dram[dname][k * 128 : (k + 1) * 128, :]
                    )

            # DMA issue order = priority: xt + wk first (unblocks K tile 0),
            # then wq, then wv. wo is issued much later.
            for k in range(kt_n):
                nc.sync.dma_start(
                    out=xt[:, k, :], in_=xt_d[k * 128 : (k + 1) * 128, :]
                )
            load_w("wk", wk_bf)
            load_w("wq", wq_bf)
            load_w("wv", wv_bf)

            def kq_group(w_bf, dst, bias_col, m, n0, pool, pool_shape, tag):
                """One K/Q projection PSUM group: 8 accumulating matmuls + evict."""
                n1 = min(n0 + 512, dst.shape[2])
                ps = pool.tile(pool_shape, FP, name="projp", tag=tag)
                for k in range(kt_n):
                    nc.tensor.matmul(
                        ps[:, : n1 - n0],
                        w_bf[:, k, m * 128 : (m + 1) * 128],
                        xt[:, k, n0:n1],
                        start=(k == 0),
                        stop=(k == kt_n - 1),
                    )
                if bias_col is not None:
                    nc.vector.tensor_scalar_add(
                        out=dst[:, m, n0:n1],
                        in0=ps[:, : n1 - n0],
                        scalar1=bias_col[:, m : m + 1],
                    )
                else:
                    nc.vector.tensor_copy(out=dst[:, m, n0:n1], in_=ps[:, : n1 - n0])

            def v_group(m, n0, pool, pool_shape, tag):
                ps = pool.tile(pool_shape, FP, name="projp", tag=tag)
                for k in range(kt_n):
                    nc.tensor.matmul(
                        ps[:, :512],
                        xt[:, k, m * 128 : (m + 1) * 128],
                        wv_bf[:, k, n0 : n0 + 512],
                        start=(k == 0),
                        stop=(k == kt_n - 1),
                    )
                dst = vsb[:, m, :].rearrange("p (a e) -> p a e", e=65)[
                    :, n0 // 64 : n0 // 64 + 8, 0:64
                ]
                src = ps[:, :512].rearrange("p (a e) -> p a e", e=64)
                if "bv" in bcast:
                    nc.vector.tensor_add(
                        out=dst,
                        in0=src,
                        in1=bcast["bv"][:, n0 : n0 + 512].rearrange(
                            "p (a e) -> p a e", e=64
                        ),
                    )
                else:
                    nc.vector.tensor_copy(out=dst, in_=src)

            # ---- upfront projections (overlap the weight DMA) ----
            with tc.tile_pool(name="projps", bufs=2, space="PSUM") as projps:
                for n0 in range(0, s, 512):
                    kq_group(
                        wk_bf, kt, bias_cols.get("bk"), 0, n0, projps, [128, 512], "projp"
                    )
                for n0 in range(0, sh, 512):
                    kq_group(
                        wq_bf, qt, bias_cols.get("bq"), 0, n0, projps, [128, 512], "projp"
                    )
                for m in range(tt_n):
                    v_group(m, 0, projps, [128, 512], "projp")

            # fill tasks, deadline in scores-iteration units (16 per head)
            tasks = []
            for m in range(1, kt_n):
                for n0 in range(0, s, 512):
                    tasks.append((32 * m, "k", m, n0))
                for n0 in range(0, sh, 512):
                    tasks.append((32 * m, "q", m, n0))
            for m in range(tt_n):
                tasks.append((8 * tt_n + m, "v", m, 512))
            tasks.sort()
            wo_issued = False
            n_v_left = tt_n

            # ---- attention, with projection fill interleaved ----
            with ExitStack() as st_att:
                psb = st_att.enter_context(tc.tile_pool(name="psb", bufs=3))
                rpool = st_att.enter_context(tc.tile_pool(name="rpool", bufs=2))
                stps = st_att.enter_context(
                    tc.tile_pool(name="stps", bufs=2, space="PSUM")
                )
                pvps = st_att.enter_context(
                    tc.tile_pool(name="pvps", bufs=2, space="PSUM")
                )
                LOOKAHEAD = 24

                def run_task(kind, fm, fn0):
                    nonlocal n_v_left, wo_issued
                    if kind == "k":
                        kq_group(
                            wk_bf, kt, bias_cols.get("bk"), fm, fn0, stps, [128, sh], "stp"
                        )
                    elif kind == "q":
                        kq_group(
                            wq_bf, qt, bias_cols.get("bq"), fm, fn0, stps, [128, sh], "stp"
                        )
                    else:
                        v_group(fm, fn0, stps, [128, sh], "stp")
                        n_v_left -= 1
                        if n_v_left == 0 and not wo_issued:
                            wo_issued = True
                            wo_tiles.append(wbuf.tile([128, kt_n, h], BF, name="wo_bf", tag="w"))
                            load_w("wo", wo_tiles[0])

                wo_tiles = []
                it = 0
                for hh in range(nh):
                    mt, po = hh // 2, 64 * (hh % 2)
                    pv = pvps.tile([65, sh], FP, name="pvp")
                    for m in range(tt_n):
                        while tasks and tasks[0][0] <= it:
                            _, kind, fm, fn0 = tasks.pop(0)
                            run_task(kind, fm, fn0)
                        if tasks and tasks[0][0] <= it + LOOKAHEAD:
                            _, kind, fm, fn0 = tasks.pop(0)
                            run_task(kind, fm, fn0)
                        stt = stps.tile([128, sh], FP, name="stp", tag="stp")
                        for c in range(0, sh, 512):
                            nc.tensor.matmul(
                                stt[:, c : c + 512],
                                kt[po : po + 64, mt, m * 128 : (m + 1) * 128],
                                qt[po : po + 64, mt, c : c + 512],
                                start=True,
                                stop=True,
                            )
                        p = psb.tile([128, sh], BF, name="pexp")
                        nc.scalar.activation(
                            p, stt, AF.Exp, bias=mask_sb[:, m : m + 1], scale=scale
                        )
                        for c in range(0, sh, 512):
                            nc.tensor.matmul(
                                pv[:, c : c + 512],
                                vsb[:, m, hh * 65 : (hh + 1) * 65],
                                p[:, c : c + 512],
                                start=(m == 0),
                                stop=(m == tt_n - 1),
                            )
                        it += 1
                    # softmax-sum eviction: roundtrip row 64 through DRAM to
                    # broadcast it across 64 partitions, then normalize there.
                    # pvps bufs=2 keeps this entirely off the PE critical path.
                    rrow = rpool.tile([1, sh], FP, name="rrow", bufs=2)
                    nc.vector.tensor_copy(out=rrow, in_=pv[64:65, :])
                    r_dram = dram.tile([sh], FP, name="rdram", tag="rdram", bufs=2)
                    nc.sync.dma_start(out=r_dram, in_=rrow)
                    rbc = rpool.tile([64, sh], FP, name="rbc", bufs=2)
                    nc.sync.dma_start(
                        out=rbc,
                        in_=bass.AP(
                            tensor=r_dram.tensor,
                            offset=r_dram.offset,
                            ap=[[0, 64], [1, sh]],
                        ),
                    )
                    rinv = rpool.tile([64, sh], FP, name="rinv", bufs=2)
                    nc.vector.reciprocal(rinv, rbc)
                    nc.vector.tensor_mul(
                        out=ctx_t[po : po + 64, mt, :],
                        in0=pv[0:64, :],
                        in1=rinv,
                    )
                for _, kind, fm, fn0 in tasks:  # leftovers (shouldn't happen)
                    run_task(kind, fm, fn0)
            wo_bf = wo_tiles[0]

            # ---- output projection + residual + LayerNorm (natural layout) ----
            with (
                tc.tile_pool(name="ops", bufs=4, space="PSUM") as ops,
                tc.tile_pool(name="osb", bufs=2) as osb,
                tc.tile_pool(name="lnp", bufs=2) as lnp,
            ):
                for m in range(qt_n):
                    pss = []
                    for n0 in range(0, h, 512):
                        ps = ops.tile([128, 512], FP, name="op")
                        # ctx_t tile mt holds heads 2mt / 2mt+1 on partitions
                        # 0-63 / 64-127, exactly matching Wo rows mt*128..(mt+1)*128,
                        # so one K=128 matmul contracts both heads at once.
                        for mt in range(nh // 2):
                            nc.tensor.matmul(
                                ps,
                                ctx_t[:, mt, m * 128 : (m + 1) * 128],
                                wo_bf[:, mt, n0 : n0 + 512],
                                start=(mt == 0),
                                stop=(mt == nh // 2 - 1),
                            )
                        pss.append((n0, ps))
                    xres = osb.tile([128, h], FP, name="xres")
                    nc.sync.dma_start(out=xres, in_=x[m * 128 : (m + 1) * 128, :])
                    o = osb.tile([128, h], FP, name="osum")
                    for n0, ps in pss:
                        nc.vector.tensor_add(
                            out=o[:, n0 : n0 + 512], in0=ps, in1=xres[:, n0 : n0 + 512]
                        )
                    if "bo" in bcast:
                        nc.vector.tensor_add(out=o, in0=o, in1=bcast["bo"])
                    nsub = (h + 511) // 512
                    stats = lnp.tile([128, nsub, 6], FP, name="stats")
                    for i in range(nsub):
                        nc.vector.bn_stats(
                            out=stats[:, i, :], in_=o[:, i * 512 : (i + 1) * 512]
                        )
                    mv = lnp.tile([128, 2], FP, name="mv")
                    nc.vector.bn_aggr(out=mv, in_=stats)
                    std = lnp.tile([128, 1], FP, name="std")
                    nc.scalar.activation(std, mv[:, 1:2], AF.Sqrt, bias=eps_sb)
                    inv = lnp.tile([128, 1], FP, name="inv")
                    nc.vector.reciprocal(inv, std)
                    y = osb.tile([128, h], FP, name="yout")
                    nc.vector.tensor_scalar(
                        out=y,
                        in0=o,
                        scalar1=mv[:, 0:1],
                        scalar2=inv,
                        op0=OP.subtract,
                        op1=OP.mult,
                    )
                    if "ln_gamma" in bcast:
                        nc.vector.tensor_mul(out=y, in0=y, in1=bcast["ln_gamma"])
                    if "ln_beta" in bcast:
                        nc.vector.tensor_add(out=y, in0=y, in1=bcast["ln_beta"])
                    nc.sync.dma_start(out=out[m * 128 : (m + 1) * 128, :], in_=y)

    _split_sync_waits(nc)
    return nc


_NC_CACHE = {}


def _get_nc(s, h, nh, sh, flags):
    key = (s, h, nh, sh, tuple(sorted(flags.items())))
    if key not in _NC_CACHE:
        _NC_CACHE[key] = _build(s, h, nh, sh, flags)
    return _NC_CACHE[key]


def _prepare(hidden_states, attention_mask, Wq, bq, Wk, bk, Wv, bv, Wo, bo, ln_gamma, ln_beta):
    hs = np.ascontiguousarray(np.asarray(hidden_states, dtype=np.float32))
    b_, s_, h_ = hs.shape
    nh_ = h_ // 64
    sh_ = s_ // 2
    am = np.asarray(attention_mask, dtype=np.float32).reshape(b_, s_)
    flags = {
        "bq": bool(np.any(np.asarray(bq))),
        "bk": bool(np.any(np.asarray(bk))),
        "bv": bool(np.any(np.asarray(bv))),
        "bo": bool(np.any(np.asarray(bo))),
        "ln_gamma": not bool(np.all(np.asarray(ln_gamma) == 1.0)),
        "ln_beta": bool(np.any(np.asarray(ln_beta))),
    }
    nc = _get_nc(s_, h_, nh_, sh_, flags)

    f32c = lambda a: np.ascontiguousarray(np.asarray(a, dtype=np.float32))
    bfc = lambda a: np.ascontiguousarray(
        np.asarray(a, dtype=np.float32).astype(ml_dtypes.bfloat16)
    )
    shared = {"wq": bfc(Wq), "wk": bfc(Wk), "wv": bfc(Wv), "wo": bfc(Wo)}
    for name, arr in (
        ("bq", bq),
        ("bk", bk),
        ("bv", bv),
        ("bo", bo),
        ("ln_gamma", ln_gamma),
        ("ln_beta", ln_beta),
    ):
        if flags[name]:
            shared[name] = f32c(arr)

    in_maps = []
    for c in range(N_CORES):
        bb, half = c // 2, c % 2
        mine = slice(half * sh_, (half + 1) * sh_)
        other = slice((1 - half) * sh_, (2 - half) * sh_)
        xp = np.concatenate([hs[bb, mine], hs[bb, other]], axis=0)
        xt = np.ascontiguousarray(xp.T.astype(ml_dtypes.bfloat16))
        mp = np.ascontiguousarray(np.concatenate([am[bb, mine], am[bb, other]]))
        in_maps.append(
            {
                "x": np.ascontiguousarray(xp[:sh_]),
                "xt": xt,
                "mask": mp,
                **shared,
            }
        )
    return nc, in_maps, (b_, s_, h_, sh_)


def _assemble(results, shape):
    b_, s_, h_, sh_ = shape
    out = np.empty((b_, s_, h_), dtype=np.float32)
    for c in range(N_CORES):
        bb, half = c // 2, c % 2
        out[bb, half * sh_ : (half + 1) * sh_] = results[c]["out"]
    return out


def kernel(**inputs) -> np.ndarray:
    nc, in_maps, shape = _prepare(**inputs)
    res = run_bass_kernel_spmd(nc, in_maps, core_ids=list(range(N_CORES)))
    return _assemble(res.results, shape)


# revision 11
# speedup vs baseline: 1.4714x; 1.0872x over previous
"""BertAttention (QKV proj + MHA + output proj + residual + LayerNorm) on 8 TRN2 NeuronCores.

Sharding: batch (4-way) x query-sequence-half (2-way) => 8 shards, no collectives.
Core c handles batch b=c//2, query half c%2. Each core computes K/V for its full
batch sequence (all heads) and Q/attention/output-proj/LayerNorm for its 1024
query rows. K/V projection work is duplicated across the 2 cores sharing a batch;
in exchange there is zero cross-core communication.

The host permutes each core's X rows so its query half comes first — attention is
permutation-invariant over keys as long as (K, V, mask) share the permutation, so
the program is identical across cores (pure SPMD) with no per-core indices.

Host pre-stages inputs: X transposed to [H, S] bf16 (feature on partitions after
DMA), weights cast to bf16, residual rows kept fp32. This removes all on-device
casts and PE transposes and halves the load DMA bytes.

Layouts (SBUF partition dim first):
  xt:      [128, H/128, S]   transposed activations, bf16 (direct DMA)
  Kt:      [128, H/128, S]   transposed keys (feature on partitions), bf16
  Qt:      [128, H/128, SH]  transposed, bf16
  V:       [128, S/128, NH*65] natural ([tok, head-dim]) with a ones column per
           head at slot 64 — the PV matmul then yields sum(exp) as row 64 for free
  scores:  St[ktok, qtok] in PSUM; softmax sum over ktok (the partition dim) comes
           from the ones-column trick; max-subtraction safely skipped (|s| <~ 1)
  ctx:     [128, NH/2, SH]   transposed (head dim on partitions), bf16
  out:     natural [qtok, H] — residual add + LayerNorm along the free dim.

Schedule: K/Q tile 0 + V chunk 0 projected upfront (~25us, overlapped with the
weight DMA), then the 16-head attention loop starts; remaining projection groups
are deadline-paced into the loop (one group per scores iteration) as PE
gap-filler. PV PSUM accumulators ping-pong across heads so the softmax-sum
eviction (DMA roundtrip broadcast + reciprocal + multiply) never stalls the PE.
"""

from contextlib import ExitStack

import numpy as np
import ml_dtypes

import bass_rust
import concourse.bass as bass
import concourse.mybir as mybir
from concourse.tile import TileContext
from concourse.bass_utils import run_bass_kernel_spmd

FP = mybir.dt.float32
BF = mybir.dt.bfloat16
E4 = mybir.dt.float8e4
DR = mybir.MatmulPerfMode.DoubleRow
AF = mybir.ActivationFunctionType
OP = mybir.AluOpType

N_CORES = 8
EPS = 1e-12

# The walrus build in this toolchain rejects instructions that carry more than
# one sync-wait command ("Too many sync wait commands", CoreV2/V3 setupSyncWait),
# while Tile freely attaches several semaphore waits to one instruction (and the
# TileContext exit drain aggregates one wait per logical processor). Hoist the
# excess waits onto standalone InstEventSemaphore carriers on the same engine,
# placed immediately before the instruction — engine streams are serial, so the
# gating semantics are identical.
_MAX_WAITS_PER_INST = 1


def _split_sync_waits(nc, cap=_MAX_WAITS_PER_INST):
    n_split = 0
    for fn in nc.m.functions:
        for bb in fn.blocks:
            insts = list(bb.instructions)
            out = []
            changed = False
            for ins in insts:
                si = ins.sync_info
                waits = list(si.on_wait) if (si is not None and si.on_wait) else []
                if len(waits) > cap:
                    head, tail = waits[: len(waits) - cap], waits[len(waits) - cap :]
                    for j, w in enumerate(head):
                        ev = mybir.InstEventSemaphore(
                            name=f"{ins.name}-sw{j}",
                            engine=ins.engine,
                            ins=[],
                            outs=[],
                            sync_info=bass_rust.SyncInfo(on_wait=[w], on_update=[]),
                        )
                        out.append(ev)
                        n_split += 1
                    si.on_wait = tail
                    changed = True
                out.append(ins)
            if changed:
                bb.instructions[:] = out
    return n_split


def _dram_row_bcast(handle, p, n):
    """AP reading DRAM vector [n] broadcast across p partitions."""
    return bass.AP(tensor=handle, offset=0, ap=[[0, p], [1, n]])


def _build(s, h, nh, sh, flags):
    """Build the per-core Bass program. flags: which bias/affine inputs matter."""
    hd = h // nh
    assert hd == 64, "head packing assumes head_dim 64 (2 heads per 128 partitions)"
    kt_n = h // 128  # contraction tiles over hidden dim
    tt_n = s // 128  # key-token tiles
    qt_n = sh // 128  # query-token tiles
    scale = 1.0 / float(np.sqrt(hd))

    nc = bass.Bass(target_bir_lowering=False)
    x = nc.dram_tensor("x", [sh, h], FP, kind="ExternalInput")  # residual rows
    xt_d = nc.dram_tensor("xt", [h, s], E4, kind="ExternalInput")  # transposed
    mask = nc.dram_tensor("mask", [s], FP, kind="ExternalInput")
    w_dram = {
        n: nc.dram_tensor(n, [h, h], E4, kind="ExternalInput")
        for n in ("wq", "wk", "wv", "wo")
    }
    vec_dram = {
        n: nc.dram_tensor(n, [h], FP, kind="ExternalInput")
        for n in ("bq", "bk", "bv", "bo", "ln_gamma", "ln_beta")
        if flags[n]
    }
    out = nc.dram_tensor("out", [sh, h], FP, kind="ExternalOutput")

    with TileContext(nc) as tc, ExitStack() as st_all:
        persist = st_all.enter_context(tc.tile_pool(name="persist", bufs=1))
        dram = st_all.enter_context(tc.tile_pool(name="dram", bufs=1, space="DRAM"))
        qt = persist.tile([128, kt_n, sh], BF)
        kt = persist.tile([128, kt_n, s], BF)
        vsb = persist.tile([128, tt_n, nh * 65], E4)
        ctx_t = persist.tile([128, nh // 2, sh], E4)
        mask_sb = persist.tile([128, tt_n], FP)
        eps_sb = persist.tile([128, 1], FP)

        nc.vector.memset(eps_sb, EPS)
        nc.sync.dma_start(out=mask_sb, in_=mask[:].rearrange("(t p) -> p t", p=128))

        # bias columns for Qt/Kt evictions (partition = output feature in tile)
        bias_cols = {}
        for name in ("bq", "bk"):
            if flags[name]:
                col = persist.tile([128, kt_n], FP, name=f"{name}_col")
                nc.sync.dma_start(
                    out=col, in_=vec_dram[name][:].rearrange("(t p) -> p t", p=128)
                )
                bias_cols[name] = col
        # rows broadcast across partitions for V/out bias and LN affine
        bcast = {}
        for name in ("bv", "bo", "ln_gamma", "ln_beta"):
            if flags[name]:
                t = persist.tile([128, h], FP, name=f"{name}_bc")
                nc.sync.dma_start(out=t, in_=_dram_row_bcast(vec_dram[name], 128, h))
                bcast[name] = t

        # ones columns in V (slot 64 of each 65-wide head block)
        for m in range(tt_n):
            v_view = vsb[:, m, :].rearrange("p (a e) -> p a e", e=65)
            nc.vector.memset(v_view[:, :, 64:65], 1.0)

        with ExitStack() as st_proj:
            xtpool = st_proj.enter_context(tc.tile_pool(name="xtpool", bufs=1))
            wbuf = st_proj.enter_context(tc.tile_pool(name="wbuf", bufs=3))

            xt = xtpool.tile([128, kt_n, s], E4)

            # slot assignment: wv takes slot 0 so the late wo load (issued
            # after the last V fill group) can reuse it; wk/wq live through
            # the whole attention loop (K/Q fill groups read them).
            wv_bf = wbuf.tile([128, kt_n, h], E4, name="wv_bf", tag="w")
            wk_bf = wbuf.tile([128, kt_n, h], E4, name="wk_bf", tag="w")
            wq_bf = wbuf.tile([128, kt_n, h], E4, name="wq_bf", tag="w")

            def load_w(dname, dst):
                for k in range(kt_n):
                    nc.sync.dma_start(
                        out=dst[:, k, :], in_=w_dram[dname][k * 128 : (k + 1) * 128, :]
                    )

            # DMA issue order = priority: xt + wk first (unblocks K tile 0),
            # then wq, then wv. wo is issued much later.
            for k in range(kt_n):
                nc.sync.dma_start(
                    out=xt[:, k, :], in_=xt_d[k * 128 : (k + 1) * 128, :]
                )
            load_w("wk", wk_bf)
            load_w("wq", wq_bf)
            load_w("wv", wv_bf)

            def kq_group(w_bf, dst, bias_col, m, n0, pool, pool_shape, tag):
                """One K/Q projection PSUM group: 8 accumulating matmuls + evict."""
                n1 = min(n0 + 512, dst.shape[2])
                ps = pool.tile(pool_shape, FP, name="projp", tag=tag)
                for k in range(0, kt_n, 2):
                    nc.tensor.matmul(
                        ps[:, : n1 - n0],
                        w_bf[:, k : k + 2, m * 128 : (m + 1) * 128],
                        xt[:, k : k + 2, n0:n1],
                        start=(k == 0),
                        stop=(k == kt_n - 2),
                        perf_mode=DR,
                    )
                if bias_col is not None:
                    nc.vector.tensor_scalar_add(
                        out=dst[:, m, n0:n1],
                        in0=ps[:, : n1 - n0],
                        scalar1=bias_col[:, m : m + 1],
                    )
                else:
                    nc.vector.tensor_copy(out=dst[:, m, n0:n1], in_=ps[:, : n1 - n0])

            def v_group(m, n0, pool, pool_shape, tag):
                ps = pool.tile(pool_shape, FP, name="projp", tag=tag)
                for k in range(0, kt_n, 2):
                    nc.tensor.matmul(
                        ps[:, :512],
                        xt[:, k : k + 2, m * 128 : (m + 1) * 128],
                        wv_bf[:, k : k + 2, n0 : n0 + 512],
                        start=(k == 0),
                        stop=(k == kt_n - 2),
                        perf_mode=DR,
                    )
                dst = vsb[:, m, :].rearrange("p (a e) -> p a e", e=65)[
                    :, n0 // 64 : n0 // 64 + 8, 0:64
                ]
                src = ps[:, :512].rearrange("p (a e) -> p a e", e=64)
                if "bv" in bcast:
                    nc.vector.tensor_add(
                        out=dst,
                        in0=src,
                        in1=bcast["bv"][:, n0 : n0 + 512].rearrange(
                            "p (a e) -> p a e", e=64
                        ),
                    )
                else:
                    nc.vector.tensor_copy(out=dst, in_=src)

            # ---- upfront projections (overlap the weight DMA) ----
            with tc.tile_pool(name="projps", bufs=2, space="PSUM") as projps:
                for n0 in range(0, s, 512):
                    kq_group(
                        wk_bf, kt, bias_cols.get("bk"), 0, n0, projps, [128, 512], "projp"
                    )
                for n0 in range(0, sh, 512):
                    kq_group(
                        wq_bf, qt, bias_cols.get("bq"), 0, n0, projps, [128, 512], "projp"
                    )
                for m in range(tt_n):
                    v_group(m, 0, projps, [128, 512], "projp")

            # fill tasks, deadline in scores-iteration units (16 per head)
            tasks = []
            for m in range(1, kt_n):
                for n0 in range(0, s, 512):
                    tasks.append((32 * m, "k", m, n0))
                for n0 in range(0, sh, 512):
                    tasks.append((32 * m, "q", m, n0))
            for m in range(tt_n):
                tasks.append((8 * tt_n + m, "v", m, 512))
            tasks.sort()
            wo_issued = False
            n_v_left = tt_n

            # ---- attention, with projection fill interleaved ----
            with ExitStack() as st_att:
                psb = st_att.enter_context(tc.tile_pool(name="psb", bufs=3))
                rpool = st_att.enter_context(tc.tile_pool(name="rpool", bufs=2))
                stps = st_att.enter_context(
                    tc.tile_pool(name="stps", bufs=2, space="PSUM")
                )
                pvps = st_att.enter_context(
                    tc.tile_pool(name="pvps", bufs=2, space="PSUM")
                )
                LOOKAHEAD = 24

                def run_task(kind, fm, fn0):
                    nonlocal n_v_left, wo_issued
                    if kind == "k":
                        kq_group(
                            wk_bf, kt, bias_cols.get("bk"), fm, fn0, stps, [128, sh], "stp"
                        )
                    elif kind == "q":
                        kq_group(
                            wq_bf, qt, bias_cols.get("bq"), fm, fn0, stps, [128, sh], "stp"
                        )
                    else:
                        v_group(fm, fn0, stps, [128, sh], "stp")
                        n_v_left -= 1
                        if n_v_left == 0 and not wo_issued:
                            wo_issued = True
                            wo_tiles.append(wbuf.tile([128, kt_n, h], E4, name="wo_bf", tag="w"))
                            load_w("wo", wo_tiles[0])

                wo_tiles = []
                it = 0
                for hh in range(nh):
                    mt, po = hh // 2, 64 * (hh % 2)
                    pv = pvps.tile([65, sh], FP, name="pvp")
                    for m in range(tt_n):
                        while tasks and tasks[0][0] <= it:
                            _, kind, fm, fn0 = tasks.pop(0)
                            run_task(kind, fm, fn0)
                        if tasks and tasks[0][0] <= it + LOOKAHEAD:
                            _, kind, fm, fn0 = tasks.pop(0)
                            run_task(kind, fm, fn0)
                        stt = stps.tile([128, sh], FP, name="stp", tag="stp")
                        for c in range(0, sh, 512):
                            nc.tensor.matmul(
                                stt[:, c : c + 512],
                                kt[po : po + 64, mt, m * 128 : (m + 1) * 128],
                                qt[po : po + 64, mt, c : c + 512],
                                start=True,
                                stop=True,
                            )
                        if m % 2 == 0:
                            p_pair = psb.tile([128, 2, sh], E4, name="pexp")
                        nc.scalar.activation(
                            p_pair[:, m % 2, :],
                            stt,
                            AF.Exp,
                            bias=mask_sb[:, m : m + 1],
                            scale=scale / 256.0,
                        )
                        if m % 2 == 1:
                            for c in range(0, sh, 512):
                                nc.tensor.matmul(
                                    pv[:, c : c + 512],
                                    vsb[:, m - 1 : m + 1, hh * 65 : (hh + 1) * 65],
                                    p_pair[:, 0:2, c : c + 512],
                                    start=(m == 1),
                                    stop=(m == tt_n - 1),
                                    perf_mode=DR,
                                )
                        it += 1
                    # softmax-sum eviction: roundtrip row 64 through DRAM to
                    # broadcast it across 64 partitions, then normalize there.
                    # pvps bufs=2 keeps this entirely off the PE critical path.
                    rrow = rpool.tile([1, sh], FP, name="rrow", bufs=2)
                    nc.vector.tensor_scalar_mul(
                        out=rrow, in0=pv[64:65, :], scalar1=0.25
                    )
                    r_dram = dram.tile([sh], FP, name="rdram", tag="rdram", bufs=2)
                    nc.sync.dma_start(out=r_dram, in_=rrow)
                    rbc = rpool.tile([64, sh], FP, name="rbc", bufs=2)
                    nc.sync.dma_start(
                        out=rbc,
                        in_=bass.AP(
                            tensor=r_dram.tensor,
                            offset=r_dram.offset,
                            ap=[[0, 64], [1, sh]],
                        ),
                    )
                    rinv = rpool.tile([64, sh], FP, name="rinv", bufs=2)
                    nc.vector.reciprocal(rinv, rbc)
                    nc.vector.tensor_mul(
                        out=ctx_t[po : po + 64, mt, :],
                        in0=pv[0:64, :],
                        in1=rinv,
                    )
                for _, kind, fm, fn0 in tasks:  # leftovers (shouldn't happen)
                    run_task(kind, fm, fn0)
            wo_bf = wo_tiles[0]

            # ---- output projection + residual + LayerNorm (natural layout) ----
            with (
                tc.tile_pool(name="ops", bufs=4, space="PSUM") as ops,
                tc.tile_pool(name="osb", bufs=2) as osb,
                tc.tile_pool(name="lnp", bufs=2) as lnp,
            ):
                for m in range(qt_n):
                    pss = []
                    for n0 in range(0, h, 512):
                        ps = ops.tile([128, 512], FP, name="op")
                        # ctx_t tile mt holds heads 2mt / 2mt+1 on partitions
                        # 0-63 / 64-127, exactly matching Wo rows mt*128..(mt+1)*128,
                        # so one K=128 matmul contracts both heads at once.
                        for mt in range(0, nh // 2, 2):
                            nc.tensor.matmul(
                                ps,
                                ctx_t[:, mt : mt + 2, m * 128 : (m + 1) * 128],
                                wo_bf[:, mt : mt + 2, n0 : n0 + 512],
                                start=(mt == 0),
                                stop=(mt == nh // 2 - 2),
                                perf_mode=DR,
                            )
                        pss.append((n0, ps))
                    xres = osb.tile([128, h], FP, name="xres")
                    nc.sync.dma_start(out=xres, in_=x[m * 128 : (m + 1) * 128, :])
                    o = osb.tile([128, h], FP, name="osum")
                    for n0, ps in pss:
                        nc.vector.tensor_add(
                            out=o[:, n0 : n0 + 512], in0=ps, in1=xres[:, n0 : n0 + 512]
                        )
                    if "bo" in bcast:
                        nc.vector.tensor_add(out=o, in0=o, in1=bcast["bo"])
                    nsub = (h + 511) // 512
                    stats = lnp.tile([128, nsub, 6], FP, name="stats")
                    for i in range(nsub):
                        nc.vector.bn_stats(
                            out=stats[:, i, :], in_=o[:, i * 512 : (i + 1) * 512]
                        )
                    mv = lnp.tile([128, 2], FP, name="mv")
                    nc.vector.bn_aggr(out=mv, in_=stats)
                    std = lnp.tile([128, 1], FP, name="std")
                    nc.scalar.activation(std, mv[:, 1:2], AF.Sqrt, bias=eps_sb)
                    inv = lnp.tile([128, 1], FP, name="inv")
                    nc.vector.reciprocal(inv, std)
                    y = osb.tile([128, h], FP, name="yout")
                    nc.vector.tensor_scalar(
                        out=y,
                        in0=o,
                        scalar1=mv[:, 0:1],
                        scalar2=inv,
                        op0=OP.subtract,
                        op1=OP.mult,
                    )
                    if "ln_gamma" in bcast:
                        nc.vector.tensor_mul(out=y, in0=y, in1=bcast["ln_gamma"])
                    if "ln_beta" in bcast:
                        nc.vector.tensor_add(out=y, in0=y, in1=bcast["ln_beta"])
                    nc.sync.dma_start(out=out[m * 128 : (m + 1) * 128, :], in_=y)

    _split_sync_waits(nc)
    return nc


_NC_CACHE = {}


def _get_nc(s, h, nh, sh, flags):
    key = (s, h, nh, sh, tuple(sorted(flags.items())))
    if key not in _NC_CACHE:
        _NC_CACHE[key] = _build(s, h, nh, sh, flags)
    return _NC_CACHE[key]


def _prepare(hidden_states, attention_mask, Wq, bq, Wk, bk, Wv, bv, Wo, bo, ln_gamma, ln_beta):
    hs = np.ascontiguousarray(np.asarray(hidden_states, dtype=np.float32))
    b_, s_, h_ = hs.shape
    nh_ = h_ // 64
    sh_ = s_ // 2
    am = np.asarray(attention_mask, dtype=np.float32).reshape(b_, s_)
    flags = {
        "bq": bool(np.any(np.asarray(bq))),
        "bk": bool(np.any(np.asarray(bk))),
        "bv": bool(np.any(np.asarray(bv))),
        "bo": bool(np.any(np.asarray(bo))),
        "ln_gamma": not bool(np.all(np.asarray(ln_gamma) == 1.0)),
        "ln_beta": bool(np.any(np.asarray(ln_beta))),
    }
    nc = _get_nc(s_, h_, nh_, sh_, flags)

    f32c = lambda a: np.ascontiguousarray(np.asarray(a, dtype=np.float32))
    f8c = lambda a, sc: np.ascontiguousarray(
        (np.asarray(a, dtype=np.float32) * sc).astype(ml_dtypes.float8_e4m3fn)
    )
    # weights x16 in fp8 (keeps small values out of the subnormal range);
    # K/Q both carry x16 so scores carry x256, folded into the Exp scale.
    # ctx_t carries x64 (x16 from V, x4 from the sum eviction), Wo x16, so
    # the out-proj PSUM carries x1024 — matched by scaling the residual
    # x1024 on the host. LayerNorm is scale-invariant, so the output is
    # unchanged.
    shared = {
        "wq": f8c(Wq, 16.0),
        "wk": f8c(Wk, 16.0),
        "wv": f8c(Wv, 16.0),
        "wo": f8c(Wo, 16.0),
    }
    scales = {"bq": 16.0, "bk": 16.0, "bv": 16.0, "bo": 1024.0}
    for name, arr in (
        ("bq", bq),
        ("bk", bk),
        ("bv", bv),
        ("bo", bo),
        ("ln_gamma", ln_gamma),
        ("ln_beta", ln_beta),
    ):
        if flags[name]:
            shared[name] = f32c(np.asarray(arr) * scales.get(name, 1.0))

    in_maps = []
    for c in range(N_CORES):
        bb, half = c // 2, c % 2
        mine = slice(half * sh_, (half + 1) * sh_)
        other = slice((1 - half) * sh_, (2 - half) * sh_)
        xp = np.concatenate([hs[bb, mine], hs[bb, other]], axis=0)
        xt = np.ascontiguousarray(xp.T.astype(ml_dtypes.float8_e4m3fn))
        mp = np.ascontiguousarray(np.concatenate([am[bb, mine], am[bb, other]]))
        in_maps.append(
            {
                "x": np.ascontiguousarray(xp[:sh_] * 1024.0),
                "xt": xt,
                "mask": mp,
                **shared,
            }
        )
    return nc, in_maps, (b_, s_, h_, sh_)


def _assemble(results, shape):
    b_, s_, h_, sh_ = shape
    out = np.empty((b_, s_, h_), dtype=np.float32)
    for c in range(N_CORES):
        bb, half = c // 2, c % 2
        out[bb, half * sh_ : (half + 1) * sh_] = results[c]["out"]
    return out


def kernel(**inputs) -> np.ndarray:
    nc, in_maps, shape = _prepare(**inputs)
    res = run_bass_kernel_spmd(nc, in_maps, core_ids=list(range(N_CORES)))
    return _assemble(res.results, shape)


# revision 13
# speedup vs baseline: 1.7334x; 1.1781x over previous
"""BertAttention (QKV proj + MHA + output proj + residual + LayerNorm) on 8 TRN2 NeuronCores.

Sharding: batch (4-way) x query-sequence-half (2-way) => 8 shards, no collectives.
Core c handles batch b=c//2, query half c%2. Each core computes K/V for its full
batch sequence (all heads) and Q/attention/output-proj/LayerNorm for its 1024
query rows. K/V projection work is duplicated across the 2 cores sharing a batch;
in exchange there is zero cross-core communication.

The host permutes each core's X rows so its query half comes first — attention is
permutation-invariant over keys as long as (K, V, mask) share the permutation, so
the program is identical across cores (pure SPMD) with no per-core indices.

Host pre-stages inputs: X transposed to [H, S] bf16 (feature on partitions after
DMA), weights cast to bf16, residual rows kept fp32. This removes all on-device
casts and PE transposes and halves the load DMA bytes.

Layouts (SBUF partition dim first):
  xt:      [128, H/128, S]   transposed activations, bf16 (direct DMA)
  Kt:      [128, H/128, S]   transposed keys (feature on partitions), bf16
  Qt:      [128, H/128, SH]  transposed, bf16
  V:       [128, S/128, NH*65] natural ([tok, head-dim]) with a ones column per
           head at slot 64 — the PV matmul then yields sum(exp) as row 64 for free
  scores:  St[ktok, qtok] in PSUM; softmax sum over ktok (the partition dim) comes
           from the ones-column trick; max-subtraction safely skipped (|s| <~ 1)
  ctx:     [128, NH/2, SH]   transposed (head dim on partitions), bf16
  out:     natural [qtok, H] — residual add + LayerNorm along the free dim.

Schedule: K/Q tile 0 + V chunk 0 projected upfront (~25us, overlapped with the
weight DMA), then the 16-head attention loop starts; remaining projection groups
are deadline-paced into the loop (one group per scores iteration) as PE
gap-filler. PV PSUM accumulators ping-pong across heads so the softmax-sum
eviction (DMA roundtrip broadcast + reciprocal + multiply) never stalls the PE.
"""

from contextlib import ExitStack

import numpy as np
import ml_dtypes

import bass_rust
import concourse.bass as bass
import concourse.mybir as mybir
from concourse.tile import TileContext
from concourse.bass_utils import run_bass_kernel_spmd

FP = mybir.dt.float32
BF = mybir.dt.bfloat16
E4 = mybir.dt.float8e4
DR = mybir.MatmulPerfMode.DoubleRow
AF = mybir.ActivationFunctionType
OP = mybir.AluOpType

N_CORES = 8
EPS = 1e-12

# The walrus build in this toolchain rejects instructions that carry more than
# one sync-wait command ("Too many sync wait commands", CoreV2/V3 setupSyncWait),
# while Tile freely attaches several semaphore waits to one instruction (and the
# TileContext exit drain aggregates one wait per logical processor). Hoist the
# excess waits onto standalone InstEventSemaphore carriers on the same engine,
# placed immediately before the instruction — engine streams are serial, so the
# gating semantics are identical.
_MAX_WAITS_PER_INST = 1


def _split_sync_waits(nc, cap=_MAX_WAITS_PER_INST):
    n_split = 0
    for fn in nc.m.functions:
        for bb in fn.blocks:
            insts = list(bb.instructions)
            out = []
            changed = False
            for ins in insts:
                si = ins.sync_info
                waits = list(si.on_wait) if (si is not None and si.on_wait) else []
                if len(waits) > cap:
                    head, tail = waits[: len(waits) - cap], waits[len(waits) - cap :]
                    for j, w in enumerate(head):
                        ev = mybir.InstEventSemaphore(
                            name=f"{ins.name}-sw{j}",
                            engine=ins.engine,
                            ins=[],
                            outs=[],
                            sync_info=bass_rust.SyncInfo(on_wait=[w], on_update=[]),
                        )
                        out.append(ev)
                        n_split += 1
                    si.on_wait = tail
                    changed = True
                out.append(ins)
            if changed:
                bb.instructions[:] = out
    return n_split


def _dram_row_bcast(handle, p, n):
    """AP reading DRAM vector [n] broadcast across p partitions."""
    return bass.AP(tensor=handle, offset=0, ap=[[0, p], [1, n]])


def _build(s, h, nh, sh, flags):
    """Build the per-core Bass program. flags: which bias/affine inputs matter."""
    hd = h // nh
    assert hd == 64, "head packing assumes head_dim 64 (2 heads per 128 partitions)"
    kt_n = h // 128  # contraction tiles over hidden dim
    tt_n = s // 128  # key-token tiles
    qt_n = sh // 128  # query-token tiles
    scale = 1.0 / float(np.sqrt(hd))

    nc = bass.Bass(target_bir_lowering=False)
    x = nc.dram_tensor("x", [sh, h], FP, kind="ExternalInput")  # residual rows
    xt_d = nc.dram_tensor("xt", [h, s], E4, kind="ExternalInput")  # transposed
    mask = nc.dram_tensor("mask", [s], FP, kind="ExternalInput")
    w_dram = {
        n: nc.dram_tensor(n, [h, h], E4, kind="ExternalInput")
        for n in ("wq", "wk", "wv", "wo")
    }
    vec_dram = {
        n: nc.dram_tensor(n, [h], FP, kind="ExternalInput")
        for n in ("bq", "bk", "bv", "bo", "ln_gamma", "ln_beta")
        if flags[n]
    }
    out = nc.dram_tensor("out", [sh, h], FP, kind="ExternalOutput")

    with TileContext(nc) as tc, ExitStack() as st_all:
        persist = st_all.enter_context(tc.tile_pool(name="persist", bufs=1))
        dram = st_all.enter_context(tc.tile_pool(name="dram", bufs=1, space="DRAM"))
        qt = persist.tile([128, kt_n, sh], BF)
        kt = persist.tile([128, kt_n, s], BF)
        vsb = persist.tile([128, tt_n, nh * 65], E4)
        ctx_t = persist.tile([128, nh // 2, sh], E4)
        mask_sb = persist.tile([128, tt_n], FP)
        eps_sb = persist.tile([128, 1], FP)

        nc.vector.memset(eps_sb, EPS)
        nc.sync.dma_start(out=mask_sb, in_=mask[:].rearrange("(t p) -> p t", p=128))

        # bias columns for Qt/Kt evictions (partition = output feature in tile)
        bias_cols = {}
        for name in ("bq", "bk"):
            if flags[name]:
                col = persist.tile([128, kt_n], FP, name=f"{name}_col")
                nc.sync.dma_start(
                    out=col, in_=vec_dram[name][:].rearrange("(t p) -> p t", p=128)
                )
                bias_cols[name] = col
        # rows broadcast across partitions for V/out bias and LN affine
        bcast = {}
        for name in ("bv", "bo", "ln_gamma", "ln_beta"):
            if flags[name]:
                t = persist.tile([128, h], FP, name=f"{name}_bc")
                nc.sync.dma_start(out=t, in_=_dram_row_bcast(vec_dram[name], 128, h))
                bcast[name] = t

        # ones columns in V (slot 64 of each 65-wide head block)
        for m in range(tt_n):
            v_view = vsb[:, m, :].rearrange("p (a e) -> p a e", e=65)
            nc.vector.memset(v_view[:, :, 64:65], 1.0)

        with ExitStack() as st_proj:
            xtpool = st_proj.enter_context(tc.tile_pool(name="xtpool", bufs=1))
            wbuf = st_proj.enter_context(tc.tile_pool(name="wbuf", bufs=3))

            xt = xtpool.tile([128, kt_n, s], E4)

            # slot assignment: wv takes slot 0 so the late wo load (issued
            # after the last V fill group) can reuse it; wk/wq live through
            # the whole attention loop (K/Q fill groups read them).
            wv_bf = wbuf.tile([128, kt_n, h], E4, name="wv_bf", tag="w")
            wk_bf = wbuf.tile([128, kt_n, h], E4, name="wk_bf", tag="w")
            wq_bf = wbuf.tile([128, kt_n, h], E4, name="wq_bf", tag="w")

            def load_w(dname, dst):
                for k in range(kt_n):
                    nc.sync.dma_start(
                        out=dst[:, k, :], in_=w_dram[dname][k * 128 : (k + 1) * 128, :]
                    )

            # DMA issue order = priority: xt + wk first (unblocks K tile 0),
            # then wq, then wv. wo is issued much later.
            for k in range(kt_n):
                nc.sync.dma_start(
                    out=xt[:, k, :], in_=xt_d[k * 128 : (k + 1) * 128, :]
                )
            load_w("wk", wk_bf)
            load_w("wq", wq_bf)
            load_w("wv", wv_bf)

            def kq_group(w_bf, dst, bias_col, m, n0, pool, pool_shape, tag):
                """One K/Q projection PSUM group: 8 accumulating matmuls + evict."""
                n1 = min(n0 + 512, dst.shape[2])
                ps = pool.tile(pool_shape, FP, name="projp", tag=tag)
                for k in range(0, kt_n, 2):
                    nc.tensor.matmul(
                        ps[:, : n1 - n0],
                        w_bf[:, k : k + 2, m * 128 : (m + 1) * 128],
                        xt[:, k : k + 2, n0:n1],
                        start=(k == 0),
                        stop=(k == kt_n - 2),
                        perf_mode=DR,
                    )
                if bias_col is not None:
                    nc.vector.tensor_scalar_add(
                        out=dst[:, m, n0:n1],
                        in0=ps[:, : n1 - n0],
                        scalar1=bias_col[:, m : m + 1],
                    )
                else:
                    nc.vector.tensor_copy(out=dst[:, m, n0:n1], in_=ps[:, : n1 - n0])

            def v_group(m, n0, pool, pool_shape, tag):
                ps = pool.tile(pool_shape, FP, name="projp", tag=tag)
                for k in range(0, kt_n, 2):
                    nc.tensor.matmul(
                        ps[:, :512],
                        xt[:, k : k + 2, m * 128 : (m + 1) * 128],
                        wv_bf[:, k : k + 2, n0 : n0 + 512],
                        start=(k == 0),
                        stop=(k == kt_n - 2),
                        perf_mode=DR,
                    )
                dst = vsb[:, m, :].rearrange("p (a e) -> p a e", e=65)[
                    :, n0 // 64 : n0 // 64 + 8, 0:64
                ]
                src = ps[:, :512].rearrange("p (a e) -> p a e", e=64)
                if "bv" in bcast:
                    nc.vector.tensor_add(
                        out=dst,
                        in0=src,
                        in1=bcast["bv"][:, n0 : n0 + 512].rearrange(
                            "p (a e) -> p a e", e=64
                        ),
                    )
                else:
                    nc.vector.tensor_copy(out=dst, in_=src)

            # ---- upfront projections (overlap the weight DMA) ----
            with tc.tile_pool(name="projps", bufs=2, space="PSUM") as projps:
                for n0 in range(0, s, 512):
                    kq_group(
                        wk_bf, kt, bias_cols.get("bk"), 0, n0, projps, [128, 512], "projp"
                    )
                for n0 in range(0, sh, 512):
                    kq_group(
                        wq_bf, qt, bias_cols.get("bq"), 0, n0, projps, [128, 512], "projp"
                    )
                for m in range(tt_n):
                    v_group(m, 0, projps, [128, 512], "projp")

            # fill tasks, deadline in scores-iteration units (16 per head)
            tasks = []
            for m in range(1, kt_n):
                for n0 in range(0, s, 512):
                    tasks.append((32 * m, "k", m, n0))
                for n0 in range(0, sh, 512):
                    tasks.append((32 * m, "q", m, n0))
            for m in range(tt_n):
                tasks.append((8 * tt_n + m, "v", m, 512))
            tasks.sort()
            n_it_total = nh * tt_n
            tasks = [
                (min(dl - 8, round((i + 0.5) * n_it_total / len(tasks))), kind, fm, fn0)
                for i, (dl, kind, fm, fn0) in enumerate(tasks)
            ]
            tasks.sort()
            wo_issued = False
            n_v_left = tt_n

            # ---- attention, with projection fill interleaved ----
            with ExitStack() as st_att:
                psb = st_att.enter_context(tc.tile_pool(name="psb", bufs=3))
                rpool = st_att.enter_context(tc.tile_pool(name="rpool", bufs=2))
                stps = st_att.enter_context(
                    tc.tile_pool(name="stps", bufs=3, space="PSUM")
                )
                pvps = st_att.enter_context(
                    tc.tile_pool(name="pvps", bufs=1, space="PSUM")
                )
                LOOKAHEAD = 24

                def run_task(kind, fm, fn0):
                    nonlocal n_v_left, wo_issued
                    if kind == "k":
                        kq_group(
                            wk_bf, kt, bias_cols.get("bk"), fm, fn0, stps, [128, sh], "stp"
                        )
                    elif kind == "q":
                        kq_group(
                            wq_bf, qt, bias_cols.get("bq"), fm, fn0, stps, [128, sh], "stp"
                        )
                    else:
                        v_group(fm, fn0, stps, [128, sh], "stp")
                        n_v_left -= 1
                        if n_v_left == 0 and not wo_issued:
                            wo_issued = True
                            wo_tiles.append(wbuf.tile([128, kt_n, h], E4, name="wo_bf", tag="w"))
                            load_w("wo", wo_tiles[0])

                wo_tiles = []
                it = 0
                for hh in range(nh):
                    mt, po = hh // 2, 64 * (hh % 2)
                    pv = pvps.tile([65, sh], FP, name="pvp")
                    for m in range(tt_n):
                        while tasks and tasks[0][0] <= it:
                            _, kind, fm, fn0 = tasks.pop(0)
                            run_task(kind, fm, fn0)
                        if tasks and tasks[0][0] <= it + LOOKAHEAD:
                            _, kind, fm, fn0 = tasks.pop(0)
                            run_task(kind, fm, fn0)
                        stt = stps.tile([128, sh], FP, name="stp", tag="stp")
                        for c in range(0, sh, 512):
                            nc.tensor.matmul(
                                stt[:, c : c + 512],
                                kt[po : po + 64, mt, m * 128 : (m + 1) * 128],
                                qt[po : po + 64, mt, c : c + 512],
                                start=True,
                                stop=True,
                            )
                        if m % 2 == 0:
                            p_pair = psb.tile([128, 2, sh], E4, name="pexp")
                        nc.scalar.activation(
                            p_pair[:, m % 2, :],
                            stt,
                            AF.Exp,
                            bias=mask_sb[:, m : m + 1],
                            scale=scale / 256.0,
                        )
                        if m % 2 == 1:
                            for c in range(0, sh, 512):
                                nc.tensor.matmul(
                                    pv[:, c : c + 512],
                                    vsb[:, m - 1 : m + 1, hh * 65 : (hh + 1) * 65],
                                    p_pair[:, 0:2, c : c + 512],
                                    start=(m == 1),
                                    stop=(m == tt_n - 1),
                                    perf_mode=DR,
                                )
                        it += 1
                    # quick-free eviction: one DVE copy releases the single PV
                    # bank; the sum-row DRAM-roundtrip broadcast, reciprocal and
                    # normalize all run from the SBUF copy, off the PE path.
                    pvc = rpool.tile([65, sh], FP, name="pvc", bufs=2)
                    nc.vector.tensor_copy(out=pvc, in_=pv)
                    rrow = rpool.tile([1, sh], FP, name="rrow", bufs=2)
                    nc.vector.tensor_scalar_mul(
                        out=rrow, in0=pvc[64:65, :], scalar1=0.25
                    )
                    r_dram = dram.tile([sh], FP, name="rdram", tag="rdram", bufs=2)
                    nc.sync.dma_start(out=r_dram, in_=rrow)
                    rbc = rpool.tile([64, sh], FP, name="rbc", bufs=2)
                    nc.sync.dma_start(
                        out=rbc,
                        in_=bass.AP(
                            tensor=r_dram.tensor,
                            offset=r_dram.offset,
                            ap=[[0, 64], [1, sh]],
                        ),
                    )
                    rinv = rpool.tile([64, sh], FP, name="rinv", bufs=2)
                    nc.vector.reciprocal(rinv, rbc)
                    nc.vector.tensor_mul(
                        out=ctx_t[po : po + 64, mt, :],
                        in0=pvc[0:64, :],
                        in1=rinv,
                    )
                for _, kind, fm, fn0 in tasks:  # leftovers (shouldn't happen)
                    run_task(kind, fm, fn0)
            wo_bf = wo_tiles[0]

            # ---- output projection + residual + LayerNorm (natural layout) ----
            with (
                tc.tile_pool(name="ops", bufs=4, space="PSUM") as ops,
                tc.tile_pool(name="osb", bufs=2) as osb,
                tc.tile_pool(name="lnp", bufs=2) as lnp,
            ):
                for m in range(qt_n):
                    pss = []
                    for n0 in range(0, h, 512):
                        ps = ops.tile([128, 512], FP, name="op")
                        # ctx_t tile mt holds heads 2mt / 2mt+1 on partitions
                        # 0-63 / 64-127, exactly matching Wo rows mt*128..(mt+1)*128,
                        # so one K=128 matmul contracts both heads at once.
                        for mt in range(0, nh // 2, 2):
                            nc.tensor.matmul(
                                ps,
                                ctx_t[:, mt : mt + 2, m * 128 : (m + 1) * 128],
                                wo_bf[:, mt : mt + 2, n0 : n0 + 512],
                                start=(mt == 0),
                                stop=(mt == nh // 2 - 2),
                                perf_mode=DR,
                            )
                        pss.append((n0, ps))
                    xres = osb.tile([128, h], FP, name="xres")
                    nc.sync.dma_start(out=xres, in_=x[m * 128 : (m + 1) * 128, :])
                    o = osb.tile([128, h], FP, name="osum")
                    for n0, ps in pss:
                        nc.vector.tensor_add(
                            out=o[:, n0 : n0 + 512], in0=ps, in1=xres[:, n0 : n0 + 512]
                        )
                    if "bo" in bcast:
                        nc.vector.tensor_add(out=o, in0=o, in1=bcast["bo"])
                    nsub = (h + 511) // 512
                    stats = lnp.tile([128, nsub, 6], FP, name="stats")
                    for i in range(nsub):
                        nc.vector.bn_stats(
                            out=stats[:, i, :], in_=o[:, i * 512 : (i + 1) * 512]
                        )
                    mv = lnp.tile([128, 2], FP, name="mv")
                    nc.vector.bn_aggr(out=mv, in_=stats)
                    std = lnp.tile([128, 1], FP, name="std")
                    nc.scalar.activation(std, mv[:, 1:2], AF.Sqrt, bias=eps_sb)
                    inv = lnp.tile([128, 1], FP, name="inv")
                    nc.vector.reciprocal(inv, std)
                    y = osb.tile([128, h], FP, name="yout")
                    nc.vector.tensor_scalar(
                        out=y,
                        in0=o,
                        scalar1=mv[:, 0:1],
                        scalar2=inv,
                        op0=OP.subtract,
                        op1=OP.mult,
                    )
                    if "ln_gamma" in bcast:
                        nc.vector.tensor_mul(out=y, in0=y, in1=bcast["ln_gamma"])
                    if "ln_beta" in bcast:
                        nc.vector.tensor_add(out=y, in0=y, in1=bcast["ln_beta"])
                    nc.sync.dma_start(out=out[m * 128 : (m + 1) * 128, :], in_=y)

    _split_sync_waits(nc)
    return nc


_NC_CACHE = {}


def _get_nc(s, h, nh, sh, flags):
    key = (s, h, nh, sh, tuple(sorted(flags.items())))
    if key not in _NC_CACHE:
        _NC_CACHE[key] = _build(s, h, nh, sh, flags)
    return _NC_CACHE[key]


def _prepare(hidden_states, attention_mask, Wq, bq, Wk, bk, Wv, bv, Wo, bo, ln_gamma, ln_beta):
    hs = np.ascontiguousarray(np.asarray(hidden_states, dtype=np.float32))
    b_, s_, h_ = hs.shape
    nh_ = h_ // 64
    sh_ = s_ // 2
    am = np.asarray(attention_mask, dtype=np.float32).reshape(b_, s_)
    flags = {
        "bq": bool(np.any(np.asarray(bq))),
        "bk": bool(np.any(np.asarray(bk))),
        "bv": bool(np.any(np.asarray(bv))),
        "bo": bool(np.any(np.asarray(bo))),
        "ln_gamma": not bool(np.all(np.asarray(ln_gamma) == 1.0)),
        "ln_beta": bool(np.any(np.asarray(ln_beta))),
    }
    nc = _get_nc(s_, h_, nh_, sh_, flags)

    f32c = lambda a: np.ascontiguousarray(np.asarray(a, dtype=np.float32))
    f8c = lambda a, sc: np.ascontiguousarray(
        (np.asarray(a, dtype=np.float32) * sc).astype(ml_dtypes.float8_e4m3fn)
    )
    # weights x16 in fp8 (keeps small values out of the subnormal range);
    # K/Q both carry x16 so scores carry x256, folded into the Exp scale.
    # ctx_t carries x64 (x16 from V, x4 from the sum eviction), Wo x16, so
    # the out-proj PSUM carries x1024 — matched by scaling the residual
    # x1024 on the host. LayerNorm is scale-invariant, so the output is
    # unchanged.
    shared = {
        "wq": f8c(Wq, 16.0),
        "wk": f8c(Wk, 16.0),
        "wv": f8c(Wv, 16.0),
        "wo": f8c(Wo, 16.0),
    }
    scales = {"bq": 16.0, "bk": 16.0, "bv": 16.0, "bo": 1024.0}
    for name, arr in (
        ("bq", bq),
        ("bk", bk),
        ("bv", bv),
        ("bo", bo),
        ("ln_gamma", ln_gamma),
        ("ln_beta", ln_beta),
    ):
        if flags[name]:
            shared[name] = f32c(np.asarray(arr) * scales.get(name, 1.0))

    in_maps = []
    for c in range(N_CORES):
        bb, half = c // 2, c % 2
        mine = slice(half * sh_, (half + 1) * sh_)
        other = slice((1 - half) * sh_, (2 - half) * sh_)
        xp = np.concatenate([hs[bb, mine], hs[bb, other]], axis=0)
        xt = np.ascontiguousarray(xp.T.astype(ml_dtypes.float8_e4m3fn))
        mp = np.ascontiguousarray(np.concatenate([am[bb, mine], am[bb, other]]))
        in_maps.append(
            {
                "x": np.ascontiguousarray(xp[:sh_] * 1024.0),
                "xt": xt,
                "mask": mp,
                **shared,
            }
        )
    return nc, in_maps, (b_, s_, h_, sh_)


def _assemble(results, shape):
    b_, s_, h_, sh_ = shape
    out = np.empty((b_, s_, h_), dtype=np.float32)
    for c in range(N_CORES):
        bb, half = c // 2, c % 2
        out[bb, half * sh_ : (half + 1) * sh_] = results[c]["out"]
    return out


def kernel(**inputs) -> np.ndarray:
    nc, in_maps, shape = _prepare(**inputs)
    res = run_bass_kernel_spmd(nc, in_maps, core_ids=list(range(N_CORES)))
    return _assemble(res.results, shape)
